# revision 1
# baseline (speedup 1.0000x reference)
"""OFA attention (dense_transformer) on 8 Trainium2 NeuronCores.

Sharding: heads split over cores (core c owns heads {2c, 2c+1}, both batches).
Per-core Bass/Tile program (see build_attention_nc below):
  phase 1 : QT/KT/VT = W_c @ hs.T (transposed projections; SCALING folded into Wq,
            c_attn folded into Wv on host; hsT DMA'd in 512-col chunks so the first
            matmul starts ~4us in; PSUM drained on ScalarE with fused bias-add
            while its exp stream hasn't started yet)
  phase 1b: V natural = PE-transpose(VT), packed [V_A | 1 | V_B | 1] bf16
  phase 2 : per (batch, 512-token t-block), streaming 128-row s-tiles:
              ST(s,t) = K Q^T           (M-split 64x64 PE tiles T0/T2/T8/T10; the
                                         two tiles of a column-pair run concurrently;
                                         per-(pair,head) PSUM tiles double-buffer so
                                         scores never serialize behind the exp)
              E = exp(ST) * expbT       (ScalarE exp PSUM -> SBUF bf16 per head;
                                         the multiply with host-precomputed
                                         exp(bias+mask) -- transposed to [s,t],
                                         bf16 -- alternates DVE/GpSimd:
                                         exp(s+b) == exp(s)*exp(b), so the bias
                                         never touches the PE and its DMA is halved)
              [O.T ; sums] += [V|1].T@E (PV matmul also produces softmax denoms;
                                         PV groups pop two-at-a-time AHEAD of the
                                         next scores to keep the PE fed)
            sums rows through DRAM into column orientation; one wide reciprocal;
            out-projection runs as M-split 64x64 tiles per head at the next
            t-block boundary; the 1/sums normalization applies at PSUM drain
            (both passes on DVE, ScalarE keeps only exp) and sums the heads.
Host: partial outputs summed over cores + bo (the "all-reduce" of the out-projection).
"""
import sys

for _p in ("/opt/trn_rl_repo",):
    if _p not in sys.path:
        sys.path.append(_p)

import numpy as np

import concourse.bass as bass
import concourse.tile as tile
from concourse import mybir
from concourse.masks import make_identity
from concourse.bass_utils import run_bass_kernel_spmd

F32 = mybir.dt.float32
BF16 = mybir.dt.bfloat16

B, T, E, NH, D = 2, 2048, 1024, 16, 64
N_CORES = 8
HPC = NH // N_CORES
DH = HPC * D
SCALING = float(D * 2.0) ** -0.5


def _waitfix(nc, limit=1):
    """This walrus build accepts at most ONE sync-wait per instruction.
    Hoist excess sem-waits onto inserted single-wait NoOps."""
    n_fixed = 0
    for bb in nc.m.functions[0].blocks:
        i = 0
        insts = bb.instructions
        while i < len(insts):
            inst = insts[i]
            si = inst.sync_info
            if si and si.on_wait and len(si.on_wait) > limit:
                extra = si.on_wait[limit:]
                si.on_wait = si.on_wait[:limit]
                for k, w in enumerate(extra):
                    nop = mybir.InstNoOp(
                        name=f"{inst.name}-waitfix{k}",
                        engine=inst.engine,
                        sync_info=mybir.SyncInfo(on_wait=[w], on_update=[]),
                        bass_nofuse=True,
                    )
                    nc.register_instruction(nop, overwrite=True)
                    insts.insert(i, nop)
                    i += 1
                n_fixed += 1
            i += 1
    return n_fixed


def build_attention_nc(B=2, T=2048, E=1024, HPC=2, D=64,
                       T_BLOCK=512, PROJ_BLOCK=512):
    """Build the per-core Bass program. Returns nc."""
    S = T
    PROJ_BLOCK = min(PROJ_BLOCK, T)
    TOK = B * T
    DH = HPC * D                      # 128
    assert DH == 128 and D == 64
    NE = E // 128                     # e-tiles
    NST = S // 128                    # s-tiles per batch
    NTB = T // T_BLOCK                # t-blocks per batch
    NJ = T_BLOCK // 128               # t-subtiles per block
    NPB = TOK // PROJ_BLOCK           # proj token blocks
    assert PROJ_BLOCK == 512

    nc = bass.Bass()

    hsT = nc.declare_dram_parameter("hsT", [E, TOK], BF16, isOutput=False)
    wqT = nc.declare_dram_parameter("wqT", [E, DH], BF16, isOutput=False)
    wkT = nc.declare_dram_parameter("wkT", [E, DH], BF16, isOutput=False)
    wvT = nc.declare_dram_parameter("wvT", [E, DH], BF16, isOutput=False)
    bq = nc.declare_dram_parameter("bq", [DH, 1], F32, isOutput=False)
    bk = nc.declare_dram_parameter("bk", [DH, 1], F32, isOutput=False)
    bv = nc.declare_dram_parameter("bv", [DH, 1], F32, isOutput=False)
    woT = nc.declare_dram_parameter("woT", [DH, E], BF16, isOutput=False)
    # exp(bias+mask) transposed to [s, t], bf16
    bias_in = nc.declare_dram_parameter("bias", [B, HPC, S, T], BF16,
                                        isOutput=False)
    out_partial = nc.declare_dram_parameter("out", [TOK, E], BF16, isOutput=True)
    rc_dram = nc.dram_tensor("rc_scratch", [B * NTB * HPC, T_BLOCK], F32)

    with tile.TileContext(nc) as tc:
        from contextlib import ExitStack
        with ExitStack() as ctx:
            consts = ctx.enter_context(tc.tile_pool(name="consts", bufs=1))
            persist = ctx.enter_context(tc.tile_pool(name="persist", bufs=1))
            expb_pool = ctx.enter_context(
                tc.tile_pool(name="expb_sb", bufs=12, space="SBUF"))

            i_bf = consts.tile([128, 128], BF16, tag="i_bf")
            make_identity(nc, i_bf[:])

            # weights: (E, DH) -> (128, NE, DH), bf16
            w_sb = {}
            for name, src in (("wq", wqT), ("wk", wkT), ("wv", wvT)):
                t = consts.tile([128, NE, DH], BF16, tag=name)
                nc.sync.dma_start(out=t[:], in_=src.rearrange("(n p) d -> p n d", p=128))
                w_sb[name] = t
            wo_sb = consts.tile([128, E], BF16, tag="wo")
            nc.sync.dma_start(out=wo_sb[:], in_=woT[:, :])
            b_sb = {}
            for name, src in (("bq", bq), ("bk", bk), ("bv", bv)):
                t = consts.tile([128, 1], F32, tag=name)
                nc.sync.dma_start(out=t[:], in_=src[:, :])
                b_sb[name] = t

            # persistent activations (QT/KT bf16; VT f32 for the PE transpose)
            QTb = [persist.tile([128, T], BF16, tag=f"QT{bb}", name=f"QT{bb}")
                   for bb in range(B)]
            KTb = [persist.tile([128, T], BF16, tag=f"KT{bb}", name=f"KT{bb}")
                   for bb in range(B)]
            VTb = [persist.tile([128, T], BF16, tag=f"VT{bb}", name=f"VT{bb}")
                   for bb in range(B)]
            V_sbb = []
            for bb in range(B):
                V_sb = persist.tile([128, T // 128, 256], BF16, tag=f"V_sb{bb}",
                                    name=f"V_sb{bb}")
                nc.vector.memset(V_sb[:, :, :], 0.0)
                nc.vector.memset(V_sb[:, :, D:D + 1], 1.0)
                nc.vector.memset(V_sb[:, :, 128 + D:128 + D + 1], 1.0)
                V_sbb.append(V_sb)

            # ---------------- phase 1: projections ----------------
            # hsT loaded in [128, 512] chunks, issued in consumption order so the
            # first matmul can start after ~1 MB instead of ~8 MB.
            with tc.tile_pool(name="hst", bufs=B * NE * (T // 512)) as hst_pool, \
                 tc.tile_pool(name="proj_ps", bufs=3, space="PSUM") as proj_ps:
                hstrips = {}
                for bb2 in range(B):
                    for c in range(T // 512):
                        for e in range(NE):
                            h = hst_pool.tile([128, 512], BF16, tag="hst",
                                              name=f"hst{bb2}_{e}_{c}")
                            nc.sync.dma_start(
                                out=h[:], in_=hsT[e * 128:(e + 1) * 128,
                                                  bb2 * T + c * 512:
                                                  bb2 * T + (c + 1) * 512])
                            hstrips[(bb2, e, c)] = h
                for pb in range(NPB):
                    t0 = pb * PROJ_BLOCK
                    bb = t0 // T
                    c = (t0 % T) // 512
                    tloc = t0 % T
                    for name, dstl in (("wq", QTb), ("wk", KTb), ("wv", VTb)):
                        ps = proj_ps.tile([128, PROJ_BLOCK], F32, tag="proj",
                                          name=f"pps{pb}_{name}")
                        for e in range(NE):
                            nc.tensor.matmul(ps[:], w_sb[name][:, e, :],
                                             hstrips[(bb, e, c)][:],
                                             start=(e == 0), stop=(e == NE - 1))
                        nc.scalar.activation(
                            out=dstl[bb][:, tloc:tloc + PROJ_BLOCK], in_=ps[:],
                            func=mybir.ActivationFunctionType.Identity,
                            bias=b_sb["b" + name[1]][:], scale=1.0)

            # ---------------- phase 1b: V natural ----------------
            with tc.tile_pool(name="vtr_ps", bufs=2, space="PSUM") as vtr_ps:
                for bb in range(B):
                    for st in range(T // 128):
                        ps = vtr_ps.tile([128, 128], BF16, tag="vtr",
                                         name=f"vtr{bb}_{st}")
                        nc.tensor.transpose(ps[:], VTb[bb][:, st * 128:(st + 1) * 128],
                                            i_bf[:])
                        nc.vector.tensor_copy(out=V_sbb[bb][:, st, 0:D],
                                              in_=ps[:, 0:D])
                        nc.vector.tensor_copy(out=V_sbb[bb][:, st, 128:128 + D],
                                              in_=ps[:, D:2 * D])

            # ---------------- phase 2: attention ----------------
            with tc.tile_pool(name="eraw_sb", bufs=3) as eraw_pool, \
                 tc.tile_pool(name="e_sb", bufs=6) as e_pool, \
                 tc.tile_pool(name="ot_sb", bufs=2) as ot_sb_pool, \
                 tc.tile_pool(name="sums", bufs=4) as sums_pool, \
                 tc.tile_pool(name="rcol", bufs=2) as rcol_pool, \
                 tc.tile_pool(name="tmp", bufs=3) as tmp_pool, \
                 tc.tile_pool(name="osb", bufs=3) as out_pool, \
                 tc.tile_pool(name="st_ps", bufs=1, space="PSUM") as st_ps, \
                 tc.tile_pool(name="ot_ps", bufs=2, space="PSUM") as ot_ps, \
                 tc.tile_pool(name="wo_ps", bufs=2, space="PSUM") as wo_ps:
                def emit_wo(pw):
                    otn_p, rcol_p, tglob_p = pw
                    for k in range(NJ):
                        os_t = out_pool.tile([128, E], BF16, tag="osb",
                                             name=f"osb{tglob_p}_{k}")
                        for n0 in range(0, E, 512):
                            nn_ = min(512, E - n0)
                            wpa = wo_ps.tile([128, 512], F32, tag="wo",
                                             name=f"wopa{tglob_p}_{k}_{n0}")
                            wpb = wo_ps.tile([128, 512], F32, tag="wo",
                                             name=f"wopb{tglob_p}_{k}_{n0}")
                            # M-split 64x64 tiles: both heads x both t-halves
                            # run concurrently in the tiled PE array
                            for m0 in (0, 64):
                                nc.tensor.matmul(
                                    wpa[m0:m0 + 64, 0:nn_],
                                    otn_p[0:D, k * 128 + m0:k * 128 + m0 + 64],
                                    wo_sb[0:D, n0:n0 + nn_],
                                    start=True, stop=True)
                                nc.tensor.matmul(
                                    wpb[m0:m0 + 64, 0:nn_],
                                    otn_p[D:2 * D, k * 128 + m0:k * 128 + m0 + 64],
                                    wo_sb[D:2 * D, n0:n0 + nn_],
                                    start=True, stop=True)
                            tmp = tmp_pool.tile([128, 512], F32, tag="tmp",
                                                name=f"tmp{tglob_p}_{k}_{n0}")
                            nc.scalar.activation(
                                out=tmp[:, 0:nn_], in_=wpa[:, 0:nn_],
                                func=mybir.ActivationFunctionType.Copy,
                                scale=rcol_p[:, 0 * NJ + k:0 * NJ + k + 1])
                            # os = (wpb * rb) + tmp in one DVE op
                            nc.vector.scalar_tensor_tensor(
                                out=os_t[:, n0:n0 + nn_], in0=wpb[:, 0:nn_],
                                scalar=rcol_p[:, 1 * NJ + k:1 * NJ + k + 1],
                                in1=tmp[:, 0:nn_],
                                op0=mybir.AluOpType.mult,
                                op1=mybir.AluOpType.add)
                        nc.gpsimd.dma_start(
                            out=out_partial[tglob_p + k * 128: tglob_p + (k + 1) * 128, :],
                            in_=os_t[:])

                gpendq = []   # (ots, batch, [(a, e_t, pst)...]) across blocks
                state = {"drain": None, "wo": None}

                def emit_pv_group(group):
                    ots_p, bb_, pend = group
                    for a, e_t, pst in pend:
                        nc.tensor.matmul(
                            ots_p[a][:],
                            V_sbb[bb_][:, pst, a * 128:a * 128 + 128],
                            e_t[:],
                            start=(pst == 0), stop=(pst == NST - 1))

                def do_drain():
                    """Drain O.T + sums of the finished block, then emit the
                    block-before-that's out-projection (its rcol is ready)."""
                    ots_p, b_, tb_, tglob_ = state["drain"]
                    state["drain"] = None
                    otn = ot_sb_pool.tile([128, T_BLOCK], BF16, tag="otn",
                                          name=f"otn{b_}_{tb_}")
                    scol = sums_pool.tile([128, HPC * NJ], F32, tag="scol",
                                          name=f"scol{b_}_{tb_}")
                    for a in range(HPC):
                        nc.vector.tensor_copy(out=otn[a * D:(a + 1) * D, :],
                                              in_=ots_p[a][0:D, :])
                        ss = sums_pool.tile([1, T_BLOCK], F32, tag="sums",
                                            name=f"sums{b_}_{tb_}_{a}")
                        nc.vector.tensor_copy(out=ss[:], in_=ots_p[a][D:D + 1, :])
                        idx = (b_ * NTB + tb_) * HPC + a
                        nc.gpsimd.dma_start(out=rc_dram[idx, :], in_=ss[:])
                        nc.gpsimd.dma_start(
                            out=scol[:, a * NJ:(a + 1) * NJ],
                            in_=rc_dram[idx, :].rearrange("(k p) -> p k", p=128))
                    rcol = rcol_pool.tile([128, HPC * NJ], F32, tag="rcol",
                                          name=f"rcol{b_}_{tb_}")
                    nc.vector.reciprocal(rcol[:], scol[:])
                    if state["wo"] is not None:
                        emit_wo(state["wo"])
                    state["wo"] = (otn, rcol, tglob_)

                for b in range(B):
                    for tb in range(NTB):
                        tglob = b * T + tb * T_BLOCK
                        # exp(bias).T tiles: [128 s, (head, s-tile) plane, T t]
                        # per s-tile-pair, both heads in one tile (planes
                        # a*2+half), on the sync HWDGE queue: FIFO behind the
                        # hsT chunks, so phase-1 DMA wins the start
                        ebs = [None] * (NST // 2)
                        for sp in range(NST // 2):
                            t = expb_pool.tile([128, 4, T_BLOCK], BF16,
                                               tag="expb",
                                               name=f"expb{b}_{tb}_{sp}")
                            r0 = sp * 256
                            for a in range(HPC):
                                nc.sync.dma_start(
                                    out=t[:, 2 * a:2 * a + 2, :],
                                    in_=bias_in[b, a, r0:r0 + 256,
                                                tb * T_BLOCK:(tb + 1) * T_BLOCK]
                                    .rearrange("(k p) t -> p k t", p=128))
                            ebs[sp] = t

                        ots = [ot_ps.tile([128, T_BLOCK], F32, tag="ot",
                                          name=f"ot{b}_{tb}_{a}") for a in range(HPC)]

                        # previous block's groups still queued (never flushed:
                        # the pend queue crosses t-block boundaries so the
                        # pipeline never drains at a boundary)
                        n_old = len(gpendq)
                        for sp in range(NST // 2):
                            # PV ahead of scores keeps the PE fed while the
                            # scores' PSUM WAR on the previous exp resolves
                            if len(gpendq) >= 4:
                                for _ in range(2):
                                    if n_old == 0 and state["drain"] is not None:
                                        do_drain()  # before any new-block PV
                                    emit_pv_group(gpendq.pop(0))
                                    if n_old:
                                        n_old -= 1
                            if n_old == 0 and state["drain"] is not None:
                                do_drain()
                            stp = st_ps.tile([128, 4, T_BLOCK], F32, tag="st",
                                             name=f"st{b}_{tb}_{sp}")
                            # M-split 64x64 tiles: (head a, out-half m0) -> PE
                            # tiles (r0, m0) = T0/T2/T8/T10
                            for half in range(2):
                                st = sp * 2 + half
                                for a in range(HPC):
                                    r0 = a * D
                                    for m0 in (0, 64):
                                        nc.tensor.matmul(
                                            stp[m0:m0 + 64, 2 * a + half, :],
                                            KTb[b][r0:r0 + D,
                                                   st * 128 + m0:st * 128 + m0 + 64],
                                            QTb[b][r0:r0 + D,
                                                   tb * T_BLOCK:tb * T_BLOCK + T_BLOCK],
                                            start=True, stop=True)
                            e_r = eraw_pool.tile([128, 4, T_BLOCK], BF16, tag="er",
                                                 name=f"er{b}_{tb}_{sp}")
                            nc.scalar.activation(
                                out=e_r[:], in_=stp[:],
                                func=mybir.ActivationFunctionType.Exp)
                            e_t = e_pool.tile([128, 4, T_BLOCK], BF16, tag="et",
                                              name=f"et{b}_{tb}_{sp}")
                            nc.vector.tensor_mul(out=e_t[:], in0=e_r[:],
                                                 in1=ebs[sp][:])
                            pend = []
                            for a in range(HPC):
                                for half in range(2):
                                    pend.append((a, e_t[:, 2 * a + half, :],
                                                 sp * 2 + half))
                            gpendq.append((ots, b, pend))
                        state["drain"] = (ots, b, tb, tglob)
                # epilogue: flush the last block's groups, then its drain + wo
                while gpendq:
                    emit_pv_group(gpendq.pop(0))
                do_drain()
                emit_wo(state["wo"])
    _waitfix(nc)
    return nc


# ---------------- host-side prep ----------------

def shard_inputs(hidden_states, attn_bias, attention_mask, Wq, bq, Wk, bk, Wv, bv,
                 Wo, bo, c_attn, n_cores=8, scaling=None):
    """Build per-core input maps. Returns (in_maps, with_mask)."""
    import ml_dtypes
    bf16 = ml_dtypes.bfloat16
    B, T, E = hidden_states.shape
    NH = c_attn.shape[0]
    D = E // NH
    HPC = NH // n_cores
    DH = HPC * D

    with_mask = bool(np.any(attention_mask))
    hsT = np.ascontiguousarray(hidden_states.reshape(B * T, E).T).astype(bf16)
    bias4 = attn_bias.reshape(B, NH, T, T)
    if with_mask:
        bias4 = bias4 + attention_mask.reshape(B, 1, T, T)

    # exp(bias+mask) as bf16 (viewed as uint16 so the per-core transpose below
    # takes numpy's fast strided-copy path)
    expb_u16 = np.exp(bias4).astype(bf16).view(np.uint16)

    if scaling is None:
        scaling = float(D * 2.0) ** -0.5

    in_maps = []
    for c in range(n_cores):
        r0 = c * DH
        sl = slice(r0, r0 + DH)
        hsl = slice(c * HPC, (c + 1) * HPC)
        cvec = np.repeat(c_attn[c * HPC:(c + 1) * HPC], D)
        m = {
            "hsT": hsT,
            "wqT": np.ascontiguousarray((Wq[sl] * scaling).T).astype(bf16),
            "wkT": np.ascontiguousarray(Wk[sl].T).astype(bf16),
            "wvT": np.ascontiguousarray((Wv[sl] * cvec[:, None]).T).astype(bf16),
            "bq": np.ascontiguousarray((bq[sl] * scaling)[:, None]).astype(np.float32),
            "bk": np.ascontiguousarray(bk[sl][:, None]).astype(np.float32),
            "bv": np.ascontiguousarray((bv[sl] * cvec)[:, None]).astype(np.float32),
            "woT": np.ascontiguousarray(Wo[:, sl].T).astype(bf16),
            # [B, HPC, S, T]: transposed exp-bias for this core's heads
            "bias": np.ascontiguousarray(
                expb_u16[:, hsl].transpose(0, 1, 3, 2)).view(bf16),
        }
        in_maps.append(m)
    return in_maps, with_mask


_NC_CACHE = {}


def run_spmd(in_maps, with_mask=False, **kwargs):
    if "nc" not in _NC_CACHE:
        _NC_CACHE["nc"] = build_attention_nc(B=B, T=T, E=E, HPC=HPC, D=D)
    nc = _NC_CACHE["nc"]
    return run_bass_kernel_spmd(nc, in_maps, list(range(N_CORES)), **kwargs)


def kernel(hidden_states, attn_bias, attention_mask, Wq, bq, Wk, bk, Wv, bv,
           Wo, bo, c_attn):
    args = [np.asarray(a, dtype=np.float32) for a in
            (hidden_states, attn_bias, attention_mask, Wq, bq, Wk, bk, Wv, bv,
             Wo, bo, c_attn)]
    (hidden_states, attn_bias, attention_mask, Wq, bq, Wk, bk, Wv, bv,
     Wo, bo, c_attn) = args
    in_maps, with_mask = shard_inputs(hidden_states, attn_bias, attention_mask,
                                      Wq, bq, Wk, bk, Wv, bv, Wo, bo, c_attn,
                                      n_cores=N_CORES, scaling=SCALING)
    res = run_spmd(in_maps, with_mask)
    out = np.zeros((B * T, E), np.float32)
    for r in res.results:
        out += r["out"]
    out += bo[None, :]
    return out.reshape(B, T, E).astype(np.float32)



# revision 10
# speedup vs baseline: 1.0904x; 1.0904x over previous
"""OFA attention (dense_transformer) on 8 Trainium2 NeuronCores — v2.

Sharding: heads split over cores (core c owns heads {2c, 2c+1}, both batches).

Per-core program (three phases, one nc, deep software pipeline):
  A: hsT-b0 DMA + Q/K proj b0 (bias folded in via ones-row matmuls; drains are
     pure DVE copies) + V-b0 computed directly in NATURAL orientation
     (stationary = hsT chunk, moving = wvT) so there is no VT / PE-transpose
     phase.  A dummy exp warms the ACT table early.
  B: attention b0 (per t-block, 8 sp-steps of 2 s-tiles x 2 heads):
       scores -> st PSUM f32 [128,4,512] (plane = 2*half+a)
       exp    -> e_r pair tile (one ACTIVATE per sp, FD=2048; ScalarE does
                 ONLY exp: exp(s+b) == exp(s)*exp(b), bias exp'd on host)
       mult   -> e_t = e_r * ebs, one DVE op per sp-PAIR (FD=4096, bf16 2x);
                 ebs tiles DMA'd as single contiguous 1MB transfers
       PV     -> [V|1].T @ E accumulates O.T + softmax sums in PSUM
     b1's Q/K/V projections are interleaved 1 task/sp under b0's attention.
     Block drains spread over the NEXT block as small tasks (no boundary
     bunching -> no PE idle window -> HAM stays warm):
       O.T+sums -> SBUF bf16; PE-transpose puts tokens into PARTITIONS, so
       1/sums applies as a per-partition DVE scalar; PE-transpose back and the
       out-projection runs heads-FUSED (contraction 128) with pure-copy drains.
       The DRAM sums-roundtrip of v1 is gone.
  C: attention b1 (same pipeline).
Host: partial outputs summed over cores + bo (the all-reduce of out_proj).

PSUM budget (8 banks): st 4 + ot 2 + aux 2 (one rotating bank-sized tag shared
by fwd/back transposes, wo matmuls and b1-proj groups).
"""
import sys
from collections import deque

for _p in ("/opt/trn_rl_repo",):
    if _p not in sys.path:
        sys.path.append(_p)

import numpy as np

import concourse.bass as bass
import concourse.tile as tile
from concourse import mybir
from concourse.masks import make_identity
from concourse.bass_utils import run_bass_kernel_spmd

F32 = mybir.dt.float32
BF16 = mybir.dt.bfloat16

B, T, E, NH, D = 2, 2048, 1024, 16, 64
N_CORES = 8
HPC = NH // N_CORES          # 2 heads per core
DH = HPC * D                 # 128
SCALING = float(D * 2.0) ** -0.5
T_BLOCK = 512
NTB = T // T_BLOCK           # 4 t-blocks per batch
NSP = 8                      # sp-steps per block (2 s-tiles each)
NST = 16                     # s-tiles per batch
NE = E // 128                # 8 e-strips
TOK = B * T


def _waitfix(nc, limit=1):
    """This walrus build accepts at most ONE sync-wait per instruction.
    Hoist excess sem-waits onto inserted single-wait NoOps."""
    n_fixed = 0
    for bb in nc.m.functions[0].blocks:
        i = 0
        insts = bb.instructions
        while i < len(insts):
            inst = insts[i]
            si = inst.sync_info
            if si and si.on_wait and len(si.on_wait) > limit:
                extra = si.on_wait[limit:]
                si.on_wait = si.on_wait[:limit]
                for k, w in enumerate(extra):
                    nop = mybir.InstNoOp(
                        name=f"{inst.name}-waitfix{k}",
                        engine=inst.engine,
                        sync_info=mybir.SyncInfo(on_wait=[w], on_update=[]),
                        bass_nofuse=True,
                    )
                    nc.register_instruction(nop, overwrite=True)
                    insts.insert(i, nop)
                    i += 1
                n_fixed += 1
            i += 1
    return n_fixed


def build_attention_nc():
    nc = bass.Bass()

    hsT = nc.declare_dram_parameter("hsT", [E, TOK], BF16, isOutput=False)
    wqT = nc.declare_dram_parameter("wqT", [E, DH], BF16, isOutput=False)
    wkT = nc.declare_dram_parameter("wkT", [E, DH], BF16, isOutput=False)
    wvT = nc.declare_dram_parameter("wvT", [E, DH], BF16, isOutput=False)
    brows = nc.declare_dram_parameter("brows", [1, 3 * DH], BF16, isOutput=False)
    woT = nc.declare_dram_parameter("woT", [DH, E], BF16, isOutput=False)
    # exp(bias+mask), pre-arranged on host to per-(block, sp-pair) tiles:
    # [B, NTB, 4 pairs, 128 p, 2 spi, 4 plane(2*half+a), 512 t]
    bias_in = nc.declare_dram_parameter("bias", [B, NTB, 4, 128, 2, 4, T_BLOCK],
                                        BF16, isOutput=False)
    out_partial = nc.declare_dram_parameter("out", [TOK, E], BF16, isOutput=True)

    with tile.TileContext(nc) as tc:
        from contextlib import ExitStack
        with ExitStack() as ctx:
            consts = ctx.enter_context(tc.tile_pool(name="consts", bufs=1))
            persist = ctx.enter_context(tc.tile_pool(name="persist", bufs=1))
            ebs_pool = ctx.enter_context(tc.tile_pool(name="ebs", bufs=3))
            er_pool = ctx.enter_context(tc.tile_pool(name="er", bufs=2))
            et_pool = ctx.enter_context(tc.tile_pool(name="et", bufs=2))
            hst_pool = ctx.enter_context(tc.tile_pool(name="hst", bufs=32))
            otn65_pool = ctx.enter_context(tc.tile_pool(name="otn65", bufs=2))
            onat_pool = ctx.enter_context(tc.tile_pool(name="onat", bufs=2))
            otnn_pool = ctx.enter_context(tc.tile_pool(name="otnn", bufs=2))
            rsb_pool = ctx.enter_context(tc.tile_pool(name="rsb", bufs=2))
            os_pool = ctx.enter_context(tc.tile_pool(name="osb", bufs=3))

            i_bf = consts.tile([128, 128], BF16, tag="i_bf")
            make_identity(nc, i_bf[:])
            ones_row = consts.tile([1, T_BLOCK], BF16, tag="ones_row")
            nc.gpsimd.memset(ones_row[:], 1.0)

            # dummy exp: pull the ACT exp table-load off the critical path
            dummy = consts.tile([128, 1], BF16, tag="dummy")
            nc.scalar.activation(out=dummy[:], in_=i_bf[:, 0:1],
                                 func=mybir.ActivationFunctionType.Exp)

            # weights: (E, DH) -> (128, NE, DH) bf16
            w_sb = {}
            for name, src in (("wq", wqT), ("wk", wkT), ("wv", wvT)):
                t = consts.tile([128, NE, DH], BF16, tag=name)
                nc.sync.dma_start(out=t[:],
                                  in_=src.rearrange("(n p) d -> p n d", p=128))
                w_sb[name] = t
            wo_sb = consts.tile([128, E], BF16, tag="wo")
            nc.sync.dma_start(out=wo_sb[:], in_=woT[:, :])
            b_rows = consts.tile([1, 3 * DH], BF16, tag="b_rows")
            nc.sync.dma_start(out=b_rows[:], in_=brows[:, :])

            # persistent activations
            QTb = [persist.tile([128, T], BF16, tag=f"QT{bb}", name=f"QT{bb}")
                   for bb in range(B)]
            KTb = [persist.tile([128, T], BF16, tag=f"KT{bb}", name=f"KT{bb}")
                   for bb in range(B)]
            V_sbb = []
            for bb in range(B):
                V_sb = persist.tile([128, NST, 256], BF16, tag=f"V_sb{bb}",
                                    name=f"V_sb{bb}")
                nc.vector.memset(V_sb[:, :, :], 0.0)
                nc.vector.memset(V_sb[:, :, D:D + 1], 1.0)
                nc.vector.memset(V_sb[:, :, 128 + D:128 + D + 1], 1.0)
                V_sbb.append(V_sb)

            # ---------- hsT strip DMA (per batch, chunk-major) ----------
            hstrips = {}

            def emit_hst_dma(bb, c, e):
                h = hst_pool.tile([128, T_BLOCK], BF16, tag="hst",
                                  name=f"hst{bb}_{c}_{e}")
                nc.sync.dma_start(
                    out=h[:], in_=hsT[e * 128:(e + 1) * 128,
                                      bb * T + c * T_BLOCK:
                                      bb * T + (c + 1) * T_BLOCK])
                hstrips[(bb, c, e)] = h

            # ---------- projection emitters (psum_pool passed in) ----------
            def emit_qk_group(pool, bb, name, dst, c):
                ps = pool.tile([128, T_BLOCK], F32, tag="aux",
                               name=f"pqk{bb}_{name}_{c}")
                for e in range(NE):
                    nc.tensor.matmul(ps[:], w_sb[name][:, e, :],
                                     hstrips[(bb, c, e)][:],
                                     start=(e == 0), stop=False)
                bi = {"wq": 0, "wk": 1}[name]
                nc.tensor.matmul(ps[:], b_rows[0:1, bi * DH:(bi + 1) * DH],
                                 ones_row[:], start=False, stop=True)
                nc.vector.tensor_copy(
                    out=dst[:, c * T_BLOCK:(c + 1) * T_BLOCK], in_=ps[:])

            def emit_v_group(pool, bb, st):
                # V natural: out [128 t, 128 dh] = sum_e hsT_chunk.T @ wvT
                c, q = st // 4, st % 4        # chunk, 128-col quarter
                ps = pool.tile([128, T_BLOCK], F32, tag="aux",
                               name=f"pv{bb}_{st}")
                for e in range(NE):
                    nc.tensor.matmul(
                        ps[:, 0:DH],
                        hstrips[(bb, c, e)][:, q * 128:(q + 1) * 128],
                        w_sb["wv"][:, e, :],
                        start=(e == 0), stop=False)
                nc.tensor.matmul(ps[:, 0:DH], ones_row[:, 0:128],
                                 b_rows[0:1, 2 * DH:3 * DH],
                                 start=False, stop=True)
                nc.vector.tensor_copy(out=V_sbb[bb][:, st, 0:D],
                                      in_=ps[:, 0:D])
                nc.vector.tensor_copy(out=V_sbb[bb][:, st, 128:128 + D],
                                      in_=ps[:, D:DH])

            # ---------- phase A: b0 projections ----------
            with tc.tile_pool(name="proj0", bufs=3, space="PSUM") as proj0:
                for e in range(NE):
                    emit_hst_dma(0, 0, e)
                emit_qk_group(proj0, 0, "wk", KTb[0], 0)
                emit_qk_group(proj0, 0, "wq", QTb[0], 0)
                for c in range(1, 4):
                    for e in range(NE):
                        emit_hst_dma(0, c, e)
                    emit_qk_group(proj0, 0, "wk", KTb[0], c)
                for st in range(NST):
                    emit_v_group(proj0, 0, st)
                for c in range(1, 4):
                    emit_qk_group(proj0, 0, "wq", QTb[0], c)

            # ---------- phases B & C: attention (+ b1 proj under B) ----------
            with tc.tile_pool(name="st_ps", bufs=1, space="PSUM") as st_ps, \
                 tc.tile_pool(name="ot_ps", bufs=2, space="PSUM") as ot_ps, \
                 tc.tile_pool(name="aux_ps", bufs=2, space="PSUM") as aux_ps:

                tasks = deque()       # small closures: drains of prev block
                proj_tasks = deque()  # b1 projection closures (phase B)
                pend_pv = deque()     # queued PV groups

                def emit_pv_group(group):
                    ots_p, bb_, e_t_t, spi_, pend = group
                    for a, half, pst in pend:
                        nc.tensor.matmul(
                            ots_p[a][:],
                            V_sbb[bb_][:, pst, a * 128:a * 128 + 128],
                            e_t_t[:, spi_, 2 * half + a, :],
                            start=(pst == 0), stop=(pst == NST - 1))

                def enqueue_block_tasks(b_, tb_, ots_):
                    tglob = b_ * T + tb_ * T_BLOCK
                    otn65 = otn65_pool.tile([128, HPC, T_BLOCK], BF16,
                                            tag="otn65", name=f"otn65_{b_}_{tb_}")
                    o_natn = onat_pool.tile([128, 8, D], BF16, tag="onat",
                                            name=f"onat_{b_}_{tb_}")
                    otn_n = otnn_pool.tile([128, T_BLOCK], BF16, tag="otnn",
                                           name=f"otnn_{b_}_{tb_}")
                    r_sb = rsb_pool.tile([128, 8], F32, tag="rsb",
                                         name=f"rsb_{b_}_{tb_}")
                    os_t = os_pool.tile([128, 4, E], BF16, tag="osb",
                                        name=f"osb_{b_}_{tb_}")

                    # otn65 copies emitted inline (all PV of this block already
                    # emitted) so later pool reuse sees the readers in order.
                    for a in range(HPC):
                        nc.vector.tensor_copy(out=otn65[0:D + 1, a, :],
                                              in_=ots_[a][0:D + 1, :])

                    def t_fwd(k, a):
                        idx = k * 2 + a
                        tr = aux_ps.tile([128, D + 1], BF16, tag="aux",
                                         name=f"trf{b_}_{tb_}_{k}_{a}")
                        nc.tensor.transpose(
                            tr[:, :],
                            otn65[0:D + 1, a, k * 128:(k + 1) * 128],
                            i_bf[0:D + 1, 0:D + 1])
                        nc.vector.reciprocal(r_sb[:, idx:idx + 1],
                                             tr[:, D:D + 1])
                        nc.vector.tensor_scalar(
                            out=o_natn[:, idx, :], in0=tr[:, 0:D],
                            scalar1=r_sb[:, idx:idx + 1], scalar2=None,
                            op0=mybir.AluOpType.mult)

                    def t_back(k):
                        trb = aux_ps.tile([128, 128], BF16, tag="aux",
                                          name=f"trb{b_}_{tb_}_{k}")
                        # [128 t, (2 a, 64 d)] -> [(a,d)=128, 128 t] in one shot
                        nc.tensor.transpose(trb[:, :],
                                            o_natn[:, k * 2:k * 2 + 2, :],
                                            i_bf[:, :])
                        nc.vector.tensor_copy(
                            out=otn_n[:, k * 128:(k + 1) * 128], in_=trb[:, :])

                    def t_wo(k, n0):
                        wp = aux_ps.tile([128, 512], F32, tag="aux",
                                         name=f"wo{b_}_{tb_}_{k}_{n0}")
                        nc.tensor.matmul(wp[:], otn_n[:, k * 128:(k + 1) * 128],
                                         wo_sb[:, n0:n0 + 512],
                                         start=True, stop=True)
                        nc.vector.tensor_copy(out=os_t[:, k, n0:n0 + 512],
                                              in_=wp[:])

                    def t_dma(k):
                        nc.gpsimd.dma_start(
                            out=out_partial[tglob + k * 128:
                                            tglob + (k + 1) * 128, :],
                            in_=os_t[:, k, :])

                    for k in range(4):
                        tasks.append(lambda k=k: t_fwd(k, 0))
                        tasks.append(lambda k=k: t_fwd(k, 1))
                        tasks.append(lambda k=k: t_back(k))
                        tasks.append(lambda k=k: t_wo(k, 0))
                        tasks.append(lambda k=k: t_wo(k, 512))
                        tasks.append(lambda k=k: t_dma(k))

                # b1 proj tasks (popped under b0 attention, phase B);
                # each entry = (#strips it needs emitted, closure)
                def enqueue_b1_proj():
                    for c in range(4):
                        need = (c + 1) * NE
                        proj_tasks.append((need,
                            lambda c=c: emit_qk_group(aux_ps, 1, "wk", KTb[1], c)))
                        proj_tasks.append((need,
                            lambda c=c: emit_qk_group(aux_ps, 1, "wq", QTb[1], c)))
                        for st in range(c * 4, c * 4 + 4):
                            proj_tasks.append((need,
                                lambda st=st: emit_v_group(aux_ps, 1, st)))
                enqueue_b1_proj()

                nb1_strips = [0]

                def emit_b1_strips(n):
                    while nb1_strips[0] < 32 and n > 0:
                        i = nb1_strips[0]
                        emit_hst_dma(1, i // NE, i % NE)
                        nb1_strips[0] += 1
                        n -= 1

                ebs_tiles = {}

                def prefetch_ebs(b_, tb_, pair_):
                    if tb_ >= NTB:
                        b_, tb_ = b_ + 1, tb_ - NTB
                    if b_ >= B or (b_, tb_, pair_) in ebs_tiles:
                        return
                    t = ebs_pool.tile([128, 2, 4, T_BLOCK], BF16, tag="ebs",
                                      name=f"ebs{b_}_{tb_}_{pair_}")
                    nc.sync.dma_start(out=t[:], in_=bias_in[b_, tb_, pair_])
                    ebs_tiles[(b_, tb_, pair_)] = t

                for pr in range(3):
                    prefetch_ebs(0, 0, pr)

                for b in range(B):
                    for tb in range(NTB):
                        ots = [ot_ps.tile([128, T_BLOCK], F32, tag="ot",
                                          name=f"ot{b}_{tb}_{a}")
                               for a in range(HPC)]
                        for pair in range(4):
                            prefetch_ebs(b, tb + (pair + 3) // 4,
                                         (pair + 3) % 4)
                            if b == 0:
                                emit_b1_strips(4)
                            ebs_t = ebs_tiles.pop((b, tb, pair))
                            e_r = er_pool.tile([128, 2, 4, T_BLOCK], BF16,
                                               tag="er", name=f"er{b}_{tb}_{pair}")
                            e_t = et_pool.tile([128, 2, 4, T_BLOCK], BF16,
                                               tag="et", name=f"et{b}_{tb}_{pair}")
                            for spi in range(2):
                                sp = pair * 2 + spi
                                st_t = st_ps.tile([128, 4, T_BLOCK], F32,
                                                  tag="st", name=f"st{b}_{tb}_{sp}")
                                for half in range(2):
                                    sti = sp * 2 + half
                                    for a in range(HPC):
                                        r0 = a * D
                                        for m0 in (0, 64):
                                            nc.tensor.matmul(
                                                st_t[m0:m0 + 64, 2 * half + a, :],
                                                KTb[b][r0:r0 + D,
                                                       sti * 128 + m0:
                                                       sti * 128 + m0 + 64],
                                                QTb[b][r0:r0 + D,
                                                       tb * T_BLOCK:
                                                       (tb + 1) * T_BLOCK],
                                                start=True, stop=True)
                                nc.scalar.activation(
                                    out=e_r[:, spi], in_=st_t[:],
                                    func=mybir.ActivationFunctionType.Exp)
                                # pop b1-proj tasks during phase B once
                                # their hsT strips have been emitted
                                if (b == 0 and tb >= 1 and proj_tasks
                                        and nb1_strips[0] >= proj_tasks[0][0]):
                                    proj_tasks.popleft()[1]()
                                # spread prev-block drain tasks
                                for _ in range(4):
                                    if tasks:
                                        tasks.popleft()()
                            nc.vector.tensor_mul(out=e_t[:], in0=e_r[:],
                                                 in1=ebs_t[:])
                            for spi in range(2):
                                sp = pair * 2 + spi
                                pend = [(a, half, sp * 2 + half)
                                        for half in range(2)
                                        for a in range(HPC)]
                                pend_pv.append((ots, b, e_t, spi, pend))
                            while len(pend_pv) > 2:
                                emit_pv_group(pend_pv.popleft())
                        # flush this block's remaining PV groups before the
                        # drain tasks reference ots (emission-order = dep-order)
                        while pend_pv:
                            emit_pv_group(pend_pv.popleft())
                        enqueue_block_tasks(b, tb, ots)
                # epilogue
                while proj_tasks:
                    proj_tasks.popleft()[1]()
                while tasks:
                    tasks.popleft()()
    _waitfix(nc)
    return nc


# ---------------- host-side prep ----------------

def shard_inputs(hidden_states, attn_bias, attention_mask, Wq, bq, Wk, bk, Wv, bv,
                 Wo, bo, c_attn, n_cores=8, scaling=None):
    """Build per-core input maps. Returns (in_maps, with_mask)."""
    import ml_dtypes
    bf16 = ml_dtypes.bfloat16
    Bb, Tt, Ee = hidden_states.shape
    NHh = c_attn.shape[0]
    Dd = Ee // NHh
    HPCc = NHh // n_cores
    DHh = HPCc * Dd

    with_mask = bool(np.any(attention_mask))
    hsT = np.ascontiguousarray(hidden_states.reshape(Bb * Tt, Ee).T).astype(bf16)
    bias4 = attn_bias.reshape(Bb, NHh, Tt, Tt)
    if with_mask:
        bias4 = bias4 + attention_mask.reshape(Bb, 1, Tt, Tt)

    if scaling is None:
        scaling = float(Dd * 2.0) ** -0.5

    expb_u16 = np.exp(bias4).astype(bf16).view(np.uint16)

    in_maps = []
    for c in range(n_cores):
        r0 = c * DHh
        sl = slice(r0, r0 + DHh)
        hsl = slice(c * HPCc, (c + 1) * HPCc)
        cvec = np.repeat(c_attn[c * HPCc:(c + 1) * HPCc], Dd)
        # ebs: [B, NTB, 4 pair, 128 p, 2 spi, 4 plane(2*half+a), 512 t]
        # value = exp(bias)[b, a, t, s] with s = (((pair*2+spi)*2+half)*128+p)
        eb = expb_u16[:, hsl]                       # [B, 2, T(t), S(s)]
        eb = eb.reshape(Bb, HPCc, NTB, T_BLOCK, 4, 2, 2, 128)
        # axes: b, a, tb, t', pair, spi, half, p -> b, tb, pair, p, spi, half, a, t'
        ebs = np.ascontiguousarray(eb.transpose(0, 2, 4, 7, 5, 6, 1, 3))
        brows = np.concatenate([(bq[sl] * scaling), bk[sl],
                                  (bv[sl] * cvec)])[None, :].astype(bf16)
        m = {
            "hsT": hsT,
            "wqT": np.ascontiguousarray((Wq[sl] * scaling).T).astype(bf16),
            "wkT": np.ascontiguousarray(Wk[sl].T).astype(bf16),
            "wvT": np.ascontiguousarray((Wv[sl] * cvec[:, None]).T).astype(bf16),
            "brows": brows,
            "woT": np.ascontiguousarray(Wo[:, sl].T).astype(bf16),
            "bias": ebs.view(bf16),
        }
        in_maps.append(m)
    return in_maps, with_mask


_NC_CACHE = {}


def run_spmd(in_maps, with_mask=False, **kwargs):
    if "nc" not in _NC_CACHE:
        _NC_CACHE["nc"] = build_attention_nc()
    nc = _NC_CACHE["nc"]
    return run_bass_kernel_spmd(nc, in_maps, list(range(N_CORES)), **kwargs)


def kernel(hidden_states, attn_bias, attention_mask, Wq, bq, Wk, bk, Wv, bv,
           Wo, bo, c_attn):
    args = [np.asarray(a, dtype=np.float32) for a in
            (hidden_states, attn_bias, attention_mask, Wq, bq, Wk, bk, Wv, bv,
             Wo, bo, c_attn)]
    (hidden_states, attn_bias, attention_mask, Wq, bq, Wk, bk, Wv, bv,
     Wo, bo, c_attn) = args
    in_maps, with_mask = shard_inputs(hidden_states, attn_bias, attention_mask,
                                      Wq, bq, Wk, bk, Wv, bv, Wo, bo, c_attn,
                                      n_cores=N_CORES, scaling=SCALING)
    res = run_spmd(in_maps, with_mask)
    out = np.zeros((B * T, E), np.float32)
    for r in res.results:
        out += r["out"]
    out += bo[None, :]
    return out.reshape(B, T, E).astype(np.float32)


# revision 11
# speedup vs baseline: 1.2471x; 1.1437x over previous
"""OFA attention (dense_transformer) on 8 Trainium2 NeuronCores — v2.

Sharding: heads split over cores (core c owns heads {2c, 2c+1}, both batches).

Per-core program (three phases, one nc, deep software pipeline):
  A: hsT-b0 DMA + Q/K proj b0 (bias folded in via ones-row matmuls; drains are
     pure DVE copies) + V-b0 computed directly in NATURAL orientation
     (stationary = hsT chunk, moving = wvT) so there is no VT / PE-transpose
     phase.  A dummy exp warms the ACT table early.
  B: attention b0 (per t-block, 8 sp-steps of 2 s-tiles x 2 heads):
       scores -> st PSUM f32 [128,4,512] (plane = 2*half+a)
       exp    -> e_r pair tile (one ACTIVATE per sp, FD=2048; ScalarE does
                 ONLY exp: exp(s+b) == exp(s)*exp(b), bias exp'd on host)
       mult   -> e_t = e_r * ebs, one DVE op per sp-PAIR (FD=4096, bf16 2x);
                 ebs tiles DMA'd as single contiguous 1MB transfers
       PV     -> [V|1].T @ E accumulates O.T + softmax sums in PSUM
     b1's Q/K/V projections are interleaved 1 task/sp under b0's attention.
     Block drains spread over the NEXT block as small tasks (no boundary
     bunching -> no PE idle window -> HAM stays warm):
       O.T+sums -> SBUF bf16; PE-transpose puts tokens into PARTITIONS, so
       1/sums applies as a per-partition DVE scalar; PE-transpose back and the
       out-projection runs heads-FUSED (contraction 128) with pure-copy drains.
       The DRAM sums-roundtrip of v1 is gone.
  C: attention b1 (same pipeline).
Host: partial outputs summed over cores + bo (the all-reduce of out_proj).

PSUM budget (8 banks): st 4 + ot 2 + aux 2 (one rotating bank-sized tag shared
by fwd/back transposes, wo matmuls and b1-proj groups).
"""
import sys
from collections import deque

for _p in ("/opt/trn_rl_repo",):
    if _p not in sys.path:
        sys.path.append(_p)

import numpy as np

import concourse.bass as bass
import concourse.tile as tile
from concourse import mybir
from concourse.masks import make_identity
from concourse.bass_utils import run_bass_kernel_spmd

F32 = mybir.dt.float32
BF16 = mybir.dt.bfloat16

B, T, E, NH, D = 2, 2048, 1024, 16, 64
N_CORES = 8
HPC = NH // N_CORES          # 2 heads per core
DH = HPC * D                 # 128
SCALING = float(D * 2.0) ** -0.5
T_BLOCK = 512
NTB = T // T_BLOCK           # 4 t-blocks per batch
NSP = 8                      # sp-steps per block (2 s-tiles each)
NST = 16                     # s-tiles per batch
NE = E // 128                # 8 e-strips
TOK = B * T


def _waitfix(nc, limit=1):
    """This walrus build accepts at most ONE sync-wait per instruction.
    Hoist excess sem-waits onto inserted single-wait NoOps."""
    n_fixed = 0
    for bb in nc.m.functions[0].blocks:
        i = 0
        insts = bb.instructions
        while i < len(insts):
            inst = insts[i]
            si = inst.sync_info
            if si and si.on_wait and len(si.on_wait) > limit:
                extra = si.on_wait[limit:]
                si.on_wait = si.on_wait[:limit]
                for k, w in enumerate(extra):
                    nop = mybir.InstNoOp(
                        name=f"{inst.name}-waitfix{k}",
                        engine=inst.engine,
                        sync_info=mybir.SyncInfo(on_wait=[w], on_update=[]),
                        bass_nofuse=True,
                    )
                    nc.register_instruction(nop, overwrite=True)
                    insts.insert(i, nop)
                    i += 1
                n_fixed += 1
            i += 1
    return n_fixed


def build_attention_nc():
    nc = bass.Bass()

    hsT = nc.declare_dram_parameter("hsT", [E, TOK], BF16, isOutput=False)
    wqT = nc.declare_dram_parameter("wqT", [E, DH], BF16, isOutput=False)
    wkT = nc.declare_dram_parameter("wkT", [E, DH], BF16, isOutput=False)
    wvT = nc.declare_dram_parameter("wvT", [E, DH], BF16, isOutput=False)
    brows = nc.declare_dram_parameter("brows", [1, 3 * DH], BF16, isOutput=False)
    woT = nc.declare_dram_parameter("woT", [DH, E], BF16, isOutput=False)
    # exp(bias+mask), pre-arranged on host to per-(block, sp-pair) tiles:
    # [B, NTB, 4 pairs, 128 p, 2 spi, 4 plane(2*half+a), 512 t]
    bias_in = nc.declare_dram_parameter("bias", [B, NTB, 4, 128, 2, 4, T_BLOCK],
                                        BF16, isOutput=False)
    out_partial = nc.declare_dram_parameter("out", [TOK, E], BF16, isOutput=True)

    with tile.TileContext(nc) as tc:
        from contextlib import ExitStack
        with ExitStack() as ctx:
            consts = ctx.enter_context(tc.tile_pool(name="consts", bufs=1))
            persist = ctx.enter_context(tc.tile_pool(name="persist", bufs=1))
            ebs_pool = ctx.enter_context(tc.tile_pool(name="ebs", bufs=3))
            er_pool = ctx.enter_context(tc.tile_pool(name="er", bufs=2))
            et_pool = ctx.enter_context(tc.tile_pool(name="et", bufs=2))
            hst_pool = ctx.enter_context(tc.tile_pool(name="hst", bufs=64))
            otn65_pool = ctx.enter_context(tc.tile_pool(name="otn65", bufs=2))
            onat_pool = ctx.enter_context(tc.tile_pool(name="onat", bufs=2))
            otnn_pool = ctx.enter_context(tc.tile_pool(name="otnn", bufs=2))
            rsb_pool = ctx.enter_context(tc.tile_pool(name="rsb", bufs=2))
            os_pool = ctx.enter_context(tc.tile_pool(name="osb", bufs=2))

            i_bf = consts.tile([128, 128], BF16, tag="i_bf")
            make_identity(nc, i_bf[:])
            ones_row = consts.tile([1, T_BLOCK], BF16, tag="ones_row")
            nc.gpsimd.memset(ones_row[:], 1.0)

            # dummy exp: pull the ACT exp table-load off the critical path
            dummy = consts.tile([128, 1], BF16, tag="dummy")
            nc.scalar.activation(out=dummy[:], in_=i_bf[:, 0:1],
                                 func=mybir.ActivationFunctionType.Exp)

            # weights: (E, DH) -> (128, NE, DH) bf16
            w_sb = {}
            for name, src in (("wq", wqT), ("wk", wkT), ("wv", wvT)):
                t = consts.tile([128, NE, DH], BF16, tag=name)
                nc.sync.dma_start(out=t[:],
                                  in_=src.rearrange("(n p) d -> p n d", p=128))
                w_sb[name] = t
            wo_sb = consts.tile([128, E], BF16, tag="wo")
            nc.sync.dma_start(out=wo_sb[:], in_=woT[:, :])
            b_rows = consts.tile([1, 3 * DH], BF16, tag="b_rows")
            nc.sync.dma_start(out=b_rows[:], in_=brows[:, :])

            # persistent activations
            QTb = [persist.tile([128, T], BF16, tag=f"QT{bb}", name=f"QT{bb}")
                   for bb in range(B)]
            KTb = [persist.tile([128, T], BF16, tag=f"KT{bb}", name=f"KT{bb}")
                   for bb in range(B)]
            V_sbb = []
            for bb in range(B):
                V_sb = persist.tile([128, NST, 256], BF16, tag=f"V_sb{bb}",
                                    name=f"V_sb{bb}")
                nc.vector.memset(V_sb[:, :, :], 0.0)
                nc.vector.memset(V_sb[:, :, D:D + 1], 1.0)
                nc.vector.memset(V_sb[:, :, 128 + D:128 + D + 1], 1.0)
                V_sbb.append(V_sb)

            # ---------- hsT strip DMA (per batch, chunk-major) ----------
            hstrips = {}

            def emit_hst_dma(bb, c, e):
                h = hst_pool.tile([128, T_BLOCK], BF16, tag="hst",
                                  name=f"hst{bb}_{c}_{e}")
                nc.sync.dma_start(
                    out=h[:], in_=hsT[e * 128:(e + 1) * 128,
                                      bb * T + c * T_BLOCK:
                                      bb * T + (c + 1) * T_BLOCK])
                hstrips[(bb, c, e)] = h

            # ---------- projection emitters (psum_pool passed in) ----------
            def emit_qk_group(pool, bb, name, dst, c):
                ps = pool.tile([128, T_BLOCK], F32, tag="aux",
                               name=f"pqk{bb}_{name}_{c}")
                for e in range(NE):
                    nc.tensor.matmul(ps[:], w_sb[name][:, e, :],
                                     hstrips[(bb, c, e)][:],
                                     start=(e == 0), stop=False)
                bi = {"wq": 0, "wk": 1}[name]
                nc.tensor.matmul(ps[:], b_rows[0:1, bi * DH:(bi + 1) * DH],
                                 ones_row[:], start=False, stop=True)
                nc.vector.tensor_copy(
                    out=dst[:, c * T_BLOCK:(c + 1) * T_BLOCK], in_=ps[:])

            def emit_v_group(pool, bb, st):
                # V natural: out [128 t, 128 dh] = sum_e hsT_chunk.T @ wvT
                c, q = st // 4, st % 4        # chunk, 128-col quarter
                ps = pool.tile([128, T_BLOCK], F32, tag="aux",
                               name=f"pv{bb}_{st}")
                for e in range(NE):
                    nc.tensor.matmul(
                        ps[:, 0:DH],
                        hstrips[(bb, c, e)][:, q * 128:(q + 1) * 128],
                        w_sb["wv"][:, e, :],
                        start=(e == 0), stop=False)
                nc.tensor.matmul(ps[:, 0:DH], ones_row[:, 0:128],
                                 b_rows[0:1, 2 * DH:3 * DH],
                                 start=False, stop=True)
                nc.vector.tensor_copy(out=V_sbb[bb][:, st, 0:D],
                                      in_=ps[:, 0:D])
                nc.vector.tensor_copy(out=V_sbb[bb][:, st, 128:128 + D],
                                      in_=ps[:, D:DH])

            # ---------- phase A: b0 K0/Q0 only (rest under tb0) ----------
            with tc.tile_pool(name="proj0", bufs=3, space="PSUM") as proj0:
                for e in range(NE):
                    emit_hst_dma(0, 0, e)
                emit_qk_group(proj0, 0, "wk", KTb[0], 0)
                emit_qk_group(proj0, 0, "wq", QTb[0], 0)
                for c in range(1, 4):
                    for e in range(NE):
                        emit_hst_dma(0, c, e)

            # ---------- phases B & C: attention (+ b1 proj under B) ----------
            with tc.tile_pool(name="st_ps", bufs=2, space="PSUM") as st_ps, \
                 tc.tile_pool(name="ot_ps", bufs=2, space="PSUM") as ot_ps, \
                 tc.tile_pool(name="aux_ps", bufs=2, space="PSUM") as aux_ps:

                tasks = deque()       # small closures: drains of prev block
                proj_tasks = deque()  # b1 projection closures (phase B)
                pend_pv = deque()     # queued PV groups

                def emit_pv_group(group):
                    ots_p, bb_, e_t_t, spi_, pend = group
                    for a, half, pst in pend:
                        nc.tensor.matmul(
                            ots_p[a][:],
                            V_sbb[bb_][:, pst, a * 128:a * 128 + 128],
                            e_t_t[:, spi_, 2 * half + a, :],
                            start=(pst == 0), stop=(pst == NST - 1))

                def enqueue_block_tasks(b_, tb_, ots_):
                    tglob = b_ * T + tb_ * T_BLOCK
                    otn65 = otn65_pool.tile([128, HPC, T_BLOCK], BF16,
                                            tag="otn65", name=f"otn65_{b_}_{tb_}")
                    o_natn = onat_pool.tile([128, 8, D], BF16, tag="onat",
                                            name=f"onat_{b_}_{tb_}")
                    otn_n = otnn_pool.tile([128, T_BLOCK], BF16, tag="otnn",
                                           name=f"otnn_{b_}_{tb_}")
                    r_sb = rsb_pool.tile([128, 8], F32, tag="rsb",
                                         name=f"rsb_{b_}_{tb_}")
                    os_t = os_pool.tile([128, 4, E], BF16, tag="osb",
                                        name=f"osb_{b_}_{tb_}")

                    # otn65 copies emitted inline (all PV of this block already
                    # emitted) so later pool reuse sees the readers in order.
                    for a in range(HPC):
                        nc.vector.tensor_copy(out=otn65[0:D + 1, a, :],
                                              in_=ots_[a][0:D + 1, :])

                    def t_fwd(k, a):
                        idx = k * 2 + a
                        tr = aux_ps.tile([128, D + 1], BF16, tag="aux",
                                         name=f"trf{b_}_{tb_}_{k}_{a}")
                        nc.tensor.transpose(
                            tr[:, :],
                            otn65[0:D + 1, a, k * 128:(k + 1) * 128],
                            i_bf[0:D + 1, 0:D + 1])
                        nc.vector.reciprocal(r_sb[:, idx:idx + 1],
                                             tr[:, D:D + 1])
                        nc.vector.tensor_scalar(
                            out=o_natn[:, idx, :], in0=tr[:, 0:D],
                            scalar1=r_sb[:, idx:idx + 1], scalar2=None,
                            op0=mybir.AluOpType.mult)

                    def t_back(k):
                        trb = aux_ps.tile([128, 128], BF16, tag="aux",
                                          name=f"trb{b_}_{tb_}_{k}")
                        # [128 t, (2 a, 64 d)] -> [(a,d)=128, 128 t] in one shot
                        nc.tensor.transpose(trb[:, :],
                                            o_natn[:, k * 2:k * 2 + 2, :],
                                            i_bf[:, :])
                        nc.vector.tensor_copy(
                            out=otn_n[:, k * 128:(k + 1) * 128], in_=trb[:, :])

                    def t_wo(k, n0):
                        wp = aux_ps.tile([128, 512], F32, tag="aux",
                                         name=f"wo{b_}_{tb_}_{k}_{n0}")
                        nc.tensor.matmul(wp[:], otn_n[:, k * 128:(k + 1) * 128],
                                         wo_sb[:, n0:n0 + 512],
                                         start=True, stop=True)
                        nc.vector.tensor_copy(out=os_t[:, k, n0:n0 + 512],
                                              in_=wp[:])

                    def t_dma(k):
                        nc.gpsimd.dma_start(
                            out=out_partial[tglob + k * 128:
                                            tglob + (k + 1) * 128, :],
                            in_=os_t[:, k, :])

                    for k in range(4):
                        tasks.append(lambda k=k: t_fwd(k, 0))
                        tasks.append(lambda k=k: t_fwd(k, 1))
                        tasks.append(lambda k=k: t_back(k))
                        tasks.append(lambda k=k: t_wo(k, 0))
                        tasks.append(lambda k=k: t_wo(k, 512))
                        tasks.append(lambda k=k: t_dma(k))

                # b1 proj tasks (popped under b0 attention, phase B);
                # each entry = (#strips it needs emitted, closure)
                def enqueue_b1_proj():
                    for c in range(4):
                        need = (c + 1) * NE
                        proj_tasks.append((need,
                            lambda c=c: emit_qk_group(aux_ps, 1, "wk", KTb[1], c)))
                        proj_tasks.append((need,
                            lambda c=c: emit_qk_group(aux_ps, 1, "wq", QTb[1], c)))
                        for st in range(c * 4, c * 4 + 4):
                            proj_tasks.append((need,
                                lambda st=st: emit_v_group(aux_ps, 1, st)))
                enqueue_b1_proj()

                b0_tasks = deque()
                for c in range(1, 4):
                    b0_tasks.append(
                        lambda c=c: emit_qk_group(aux_ps, 0, "wk", KTb[0], c))
                for st in range(NST):
                    b0_tasks.append(lambda st=st: emit_v_group(aux_ps, 0, st))
                for c in range(1, 4):
                    b0_tasks.append(
                        lambda c=c: emit_qk_group(aux_ps, 0, "wq", QTb[0], c))

                nb1_strips = [0]

                def emit_b1_strips(n):
                    while nb1_strips[0] < 32 and n > 0:
                        i = nb1_strips[0]
                        emit_hst_dma(1, i // NE, i % NE)
                        nb1_strips[0] += 1
                        n -= 1

                ebs_tiles = {}

                def prefetch_ebs(b_, tb_, pair_):
                    if tb_ >= NTB:
                        b_, tb_ = b_ + 1, tb_ - NTB
                    if b_ >= B or (b_, tb_, pair_) in ebs_tiles:
                        return
                    t = ebs_pool.tile([128, 2, 4, T_BLOCK], BF16, tag="ebs",
                                      name=f"ebs{b_}_{tb_}_{pair_}")
                    nc.sync.dma_start(out=t[:], in_=bias_in[b_, tb_, pair_])
                    ebs_tiles[(b_, tb_, pair_)] = t

                for pr in range(3):
                    prefetch_ebs(0, 0, pr)

                for b in range(B):
                    for tb in range(NTB):
                        ots = [ot_ps.tile([128, T_BLOCK], F32, tag="ot",
                                          name=f"ot{b}_{tb}_{a}")
                               for a in range(HPC)]
                        for pair in range(4):
                            prefetch_ebs(b, tb + (pair + 3) // 4,
                                         (pair + 3) % 4)
                            if b == 0:
                                emit_b1_strips(4)
                            ebs_t = ebs_tiles.pop((b, tb, pair))
                            e_r = er_pool.tile([128, 2, 4, T_BLOCK], BF16,
                                               tag="er", name=f"er{b}_{tb}_{pair}")
                            e_t = et_pool.tile([128, 2, 4, T_BLOCK], BF16,
                                               tag="et", name=f"et{b}_{tb}_{pair}")
                            for spi in range(2):
                                sp = pair * 2 + spi
                                # side work first: PE chews ready ops while
                                # the next scores' WAR resolves
                                if b == 0:
                                    for _ in range(3):
                                        if b0_tasks:
                                            b0_tasks.popleft()()
                                if (b == 0 and tb >= 1 and proj_tasks
                                        and nb1_strips[0] >= proj_tasks[0][0]):
                                    proj_tasks.popleft()[1]()
                                for _ in range(4):
                                    if tasks:
                                        tasks.popleft()()
                                for half in range(2):
                                    sti = sp * 2 + half
                                    st_t = st_ps.tile([128, 2, T_BLOCK], F32,
                                                      tag="st",
                                                      name=f"st{b}_{tb}_{sti}")
                                    for a in range(HPC):
                                        r0 = a * D
                                        for m0 in (0, 64):
                                            nc.tensor.matmul(
                                                st_t[m0:m0 + 64, a, :],
                                                KTb[b][r0:r0 + D,
                                                       sti * 128 + m0:
                                                       sti * 128 + m0 + 64],
                                                QTb[b][r0:r0 + D,
                                                       tb * T_BLOCK:
                                                       (tb + 1) * T_BLOCK],
                                                start=True, stop=True)
                                    nc.scalar.activation(
                                        out=e_r[:, spi, 2 * half:2 * half + 2, :],
                                        in_=st_t[:],
                                        func=mybir.ActivationFunctionType.Exp)
                            nc.vector.tensor_mul(out=e_t[:], in0=e_r[:],
                                                 in1=ebs_t[:])
                            for spi in range(2):
                                sp = pair * 2 + spi
                                pend = [(a, half, sp * 2 + half)
                                        for half in range(2)
                                        for a in range(HPC)]
                                pend_pv.append((ots, b, e_t, spi, pend))
                            while len(pend_pv) > 4:
                                emit_pv_group(pend_pv.popleft())
                        # flush this block's remaining PV groups before the
                        # drain tasks reference ots (emission-order = dep-order)
                        while pend_pv:
                            emit_pv_group(pend_pv.popleft())
                        enqueue_block_tasks(b, tb, ots)
                # epilogue
                while b0_tasks:
                    b0_tasks.popleft()()
                while proj_tasks:
                    proj_tasks.popleft()[1]()
                while tasks:
                    tasks.popleft()()
    _waitfix(nc)
    return nc


# ---------------- host-side prep ----------------

def shard_inputs(hidden_states, attn_bias, attention_mask, Wq, bq, Wk, bk, Wv, bv,
                 Wo, bo, c_attn, n_cores=8, scaling=None):
    """Build per-core input maps. Returns (in_maps, with_mask)."""
    import ml_dtypes
    bf16 = ml_dtypes.bfloat16
    Bb, Tt, Ee = hidden_states.shape
    NHh = c_attn.shape[0]
    Dd = Ee // NHh
    HPCc = NHh // n_cores
    DHh = HPCc * Dd

    with_mask = bool(np.any(attention_mask))
    hsT = np.ascontiguousarray(hidden_states.reshape(Bb * Tt, Ee).T).astype(bf16)
    bias4 = attn_bias.reshape(Bb, NHh, Tt, Tt)
    if with_mask:
        bias4 = bias4 + attention_mask.reshape(Bb, 1, Tt, Tt)

    if scaling is None:
        scaling = float(Dd * 2.0) ** -0.5

    expb_u16 = np.exp(bias4).astype(bf16).view(np.uint16)

    in_maps = []
    for c in range(n_cores):
        r0 = c * DHh
        sl = slice(r0, r0 + DHh)
        hsl = slice(c * HPCc, (c + 1) * HPCc)
        cvec = np.repeat(c_attn[c * HPCc:(c + 1) * HPCc], Dd)
        # ebs: [B, NTB, 4 pair, 128 p, 2 spi, 4 plane(2*half+a), 512 t]
        # value = exp(bias)[b, a, t, s] with s = (((pair*2+spi)*2+half)*128+p)
        eb = expb_u16[:, hsl]                       # [B, 2, T(t), S(s)]
        eb = eb.reshape(Bb, HPCc, NTB, T_BLOCK, 4, 2, 2, 128)
        # axes: b, a, tb, t', pair, spi, half, p -> b, tb, pair, p, spi, half, a, t'
        ebs = np.ascontiguousarray(eb.transpose(0, 2, 4, 7, 5, 6, 1, 3))
        brows = np.concatenate([(bq[sl] * scaling), bk[sl],
                                  (bv[sl] * cvec)])[None, :].astype(bf16)
        m = {
            "hsT": hsT,
            "wqT": np.ascontiguousarray((Wq[sl] * scaling).T).astype(bf16),
            "wkT": np.ascontiguousarray(Wk[sl].T).astype(bf16),
            "wvT": np.ascontiguousarray((Wv[sl] * cvec[:, None]).T).astype(bf16),
            "brows": brows,
            "woT": np.ascontiguousarray(Wo[:, sl].T).astype(bf16),
            "bias": ebs.view(bf16),
        }
        in_maps.append(m)
    return in_maps, with_mask


_NC_CACHE = {}


def run_spmd(in_maps, with_mask=False, **kwargs):
    if "nc" not in _NC_CACHE:
        _NC_CACHE["nc"] = build_attention_nc()
    nc = _NC_CACHE["nc"]
    return run_bass_kernel_spmd(nc, in_maps, list(range(N_CORES)), **kwargs)


def kernel(hidden_states, attn_bias, attention_mask, Wq, bq, Wk, bk, Wv, bv,
           Wo, bo, c_attn):
    args = [np.asarray(a, dtype=np.float32) for a in
            (hidden_states, attn_bias, attention_mask, Wq, bq, Wk, bk, Wv, bv,
             Wo, bo, c_attn)]
    (hidden_states, attn_bias, attention_mask, Wq, bq, Wk, bk, Wv, bv,
     Wo, bo, c_attn) = args
    in_maps, with_mask = shard_inputs(hidden_states, attn_bias, attention_mask,
                                      Wq, bq, Wk, bk, Wv, bv, Wo, bo, c_attn,
                                      n_cores=N_CORES, scaling=SCALING)
    res = run_spmd(in_maps, with_mask)
    out = np.zeros((B * T, E), np.float32)
    for r in res.results:
        out += r["out"]
    out += bo[None, :]
    return out.reshape(B, T, E).astype(np.float32)
